# revision 1
# baseline (speedup 1.0000x reference)
"""Trainium2 Bass kernel for DebiasSoftConLoss (SupCon-style loss with
confidence-weighted mask), 8-way row-sharded.

Math (forward only; B=4096, V=2, D=128, N=V*B=8192, T=0.07):
  C = cat(unbind(features,1))           # [N, D], L2-normalized rows
  dot[i,j] = C[i]·C[j]                  # logits = dot / T
  Row max of logits is attained on the diagonal (dot[i,i]=1, off-diag << 1),
  and log_prob is shift-invariant, so we shift by dot[i,i]/T.  The anchor is
  fed to both the PE and the DVE in bf16 so the computed diagonal matches the
  matmul's diagonal arithmetic and the self-term drops out exactly.
  denom_i  = sum_j exp((dot[i,j]-dot[i,i])/T) - 1          (drop self term)
  L_i      = log(denom_i + 1e-9)
  mask[i,j]= mp_i * mp_j * [lab_i == lab_j] * [i != j]     (lab/mp tiled to N)
  s2_i     = sum_j mask[i,j]           = mp_i * (S_{lab_i} - mp_i)
  s1_i     = sum_j mask[i,j] * (dot[i,j]-dot[i,i])/T
           = mp_i * (C[i]·g_{lab_i} - dot[i,i]*S_{lab_i}) / T
  where S_c = sum_{lab_j=c} mp_j and g_c = sum_{lab_j=c} mp_j C[j]  (class
  sums; the self term cancels inside s1 and is absent from denom_i).
  loss_i   = (L_i*s2_i - s1_i) / (s2_i if s2_i != 0 else 1)
  out      = mean_i loss_i

Only the softmax denominators need O(N^2) work; everything else collapses to
tiny per-class matmuls.  Per-core: 1024 rows x 8192 cols of exp on the Scalar
(ACT) engine is the roofline; bf16 matmuls run underneath, and the exp row
sums are split between ACT's accumulator and DVE tensor_reduce.
"""

import numpy as np

B = 4096
V = 2
D = 128
N = B * V
CORES = 8
RPC = N // CORES          # rows per core = 1024
RT = RPC // 128           # row tiles per core = 8
CHUNKS = N // 128         # 64 column chunks of 128
NCLS = 10                 # label values are 0..9
GW = 2048                 # column group width for the exp pass
MG = N // GW              # column groups per row tile = 4
TEMP = 0.07
INVT = 1.0 / TEMP
EPS = 1e-9

_CACHE = {}


def _build_program():
    import concourse.bass as bass
    import concourse.tile as tile
    from concourse import bacc, mybir
    from concourse.bass import ds, ts

    f32 = mybir.dt.float32
    bf16 = mybir.dt.bfloat16
    AF = mybir.ActivationFunctionType
    OP = mybir.AluOpType

    nc = bacc.Bacc(None, target_bir_lowering=False)

    ct_d = nc.dram_tensor("ct", [128, N], bf16, kind="ExternalInput")
    crm_d = nc.dram_tensor("crm", [128, CHUNKS * (D + 1)], bf16, kind="ExternalInput")
    anct_d = nc.dram_tensor("anct", [128, RPC], bf16, kind="ExternalInput")
    anc_d = nc.dram_tensor("anc", [128, RPC], bf16, kind="ExternalInput")
    mpr_d = nc.dram_tensor("mpr", [128, RT], f32, kind="ExternalInput")
    labr_d = nc.dram_tensor("labr", [RPC], f32, kind="ExternalInput")
    labj_d = nc.dram_tensor("labj", [128, CHUNKS], f32, kind="ExternalInput")
    mpj_d = nc.dram_tensor("mpj", [128, CHUNKS], f32, kind="ExternalInput")
    loss_d = nc.dram_tensor("loss", [128, RT], f32, kind="ExternalOutput")

    with tile.TileContext(nc) as tc:
        with (
            tc.tile_pool(name="big", bufs=1) as big,
            tc.tile_pool(name="sm", bufs=1) as sm,
            tc.tile_pool(name="scr", bufs=2) as scr,
            tc.tile_pool(name="ps", bufs=2, space="PSUM") as ps,
        ):
            # ---- input DMAs; critical-path ones first (Sync ring is FIFO
            # and each issue costs ~0.65us of queue time) ----
            sb_anct = sm.tile([128, RPC], bf16)
            nc.sync.dma_start(out=sb_anct[:, 0:128], in_=anct_d[:, 0:128])
            sb_ct = big.tile([128, N], bf16)
            nc.sync.dma_start(out=sb_ct[:, 0:512], in_=ct_d[:, 0:512])
            sb_anc = sm.tile([128, RPC], bf16)
            nc.sync.dma_start(out=sb_anc[:, :], in_=anc_d[:, :])
            for q in range(1, 4):
                nc.sync.dma_start(
                    out=sb_ct[:, ts(q, 512)], in_=ct_d[:, ts(q, 512)]
                )
            nc.sync.dma_start(out=sb_anct[:, 128:RPC], in_=anct_d[:, 128:RPC])
            nc.sync.dma_start(out=sb_ct[:, 2048:4096], in_=ct_d[:, 2048:4096])
            nc.sync.dma_start(out=sb_ct[:, 4096:6144], in_=ct_d[:, 4096:6144])
            nc.sync.dma_start(out=sb_ct[:, 6144:8192], in_=ct_d[:, 6144:8192])
            sb_mpr = sm.tile([128, RT], f32)
            nc.sync.dma_start(out=sb_mpr[:, :], in_=mpr_d[:, :])
            # contrast row-major + ones column [j, d|1], j on partitions
            sb_crm = big.tile([128, CHUNKS * (D + 1)], bf16)
            W2 = CHUNKS * (D + 1) // 2
            nc.sync.dma_start(out=sb_crm[:, 0:W2], in_=crm_d[:, 0:W2])
            nc.sync.dma_start(out=sb_crm[:, W2:], in_=crm_d[:, W2:])
            sb_labj = sm.tile([128, CHUNKS], f32)
            nc.sync.dma_start(out=sb_labj[:, :], in_=labj_d[:, :])
            sb_mpj = sm.tile([128, CHUNKS], f32)
            nc.sync.dma_start(out=sb_mpj[:, :], in_=mpj_d[:, :])
            # row labels broadcast across the first NCLS partitions (SWDGE)
            sb_labrep = sm.tile([NCLS, RPC], f32)
            labr_ap = labr_d[:]
            labr_b = bass.AP(
                tensor=labr_ap.tensor,
                offset=labr_ap.offset,
                ap=[[0, NCLS]] + list(labr_ap.ap),
            )
            nc.gpsimd.dma_start(out=sb_labrep[:, :], in_=labr_b)

            # ---- per-row-tile exp biases, hoisted off the critical loop ----
            dii = sm.tile([128, RT], f32)       # dot[i,i] (bf16 inputs)
            negb = sm.tile([128, RT], f32)      # -dot[i,i]/T  (exp bias)
            for t in range(RT):
                sq = scr.tile([128, 128], f32, tag="sq")
                nc.vector.scalar_tensor_tensor(
                    out=sq[:, :],
                    in0=sb_anc[:, ts(t, 128)],
                    scalar=0.0,
                    in1=sb_anc[:, ts(t, 128)],
                    op0=OP.add,
                    op1=OP.mult,
                    accum_out=dii[:, t : t + 1],
                )
                nc.vector.tensor_scalar(
                    negb[:, t : t + 1], dii[:, t : t + 1], -INVT, None, OP.mult
                )

            # ---- tiny device-side prep (all off the critical path) ----
            iota_i = sm.tile([NCLS, 1], mybir.dt.int32)
            nc.gpsimd.iota(iota_i[:, :], pattern=[[0, 1]], base=0, channel_multiplier=1)
            iota_f = sm.tile([NCLS, 1], f32)
            nc.vector.tensor_copy(out=iota_f[:, :], in_=iota_i[:, :])

            # one-hot^T of this core's row labels: [c, i] = (lab_i == c)
            onehotT = sm.tile([NCLS, RPC], bf16)
            nc.vector.tensor_scalar(
                onehotT[:, :], sb_labrep[:, :], iota_f[:, :], None, OP.is_equal
            )

            # Woh[j-part, chunk, c] = mp_j * (lab_j == c)
            woh = sm.tile([128, CHUNKS, NCLS], bf16)
            for c in range(NCLS):
                nc.vector.scalar_tensor_tensor(
                    out=woh[:, :, c],
                    in0=sb_labj[:, :],
                    scalar=float(c),
                    in1=sb_mpj[:, :],
                    op0=OP.is_equal,
                    op1=OP.mult,
                )

            qcol = sm.tile([128, RT], f32)      # C[i]·g_{lab_i} / T
            scol = sm.tile([128, RT], f32)      # S_{lab_i}
            dsum = sm.tile([128, RT, MG], f32)  # partial exp row sums
            g_sb = sm.tile([NCLS, D + 1], bf16)  # [g/T | S]
            gall = sm.tile([128, RT * (D + 1)], f32)

            def emit_g_phase():
                # g_aug[c, :] = sum_j mp_j [lab_j=c] * [C[j,:] | 1]
                gps = ps.tile([NCLS, D + 1], f32, tag="ps")
                for k in range(CHUNKS):
                    nc.tensor.matmul(
                        gps[:, :],
                        lhsT=woh[:, k, :],
                        rhs=sb_crm[:, ds(k * (D + 1), D + 1)],
                        start=(k == 0),
                        stop=(k == CHUNKS - 1),
                    )
                nc.vector.tensor_scalar(
                    g_sb[:, 0:D], gps[:, 0:D], INVT, None, OP.mult
                )
                nc.vector.tensor_copy(out=g_sb[:, D : D + 1], in_=gps[:, D : D + 1])

            def emit_G_phase(half):
                # [q*T | S] per row, 4 row tiles per call: PSUM slots padded
                # to 256 so no matmul output straddles a bank; one strided
                # copy to SBUF per half so the PSUM slot frees fast.
                H = RT // 2
                t0h = half * H
                gt = ps.tile([128, H, 256], f32, tag="ps")
                for t in range(H):
                    nc.tensor.matmul(
                        gt[:, t, 0 : D + 1],
                        lhsT=onehotT[:, ts(t0h + t, 128)],
                        rhs=g_sb[:, :],
                        start=True,
                        stop=True,
                    )
                nc.vector.tensor_copy(
                    out=gall[:, ds(t0h * (D + 1), H * (D + 1))],
                    in_=gt[:, :, 0 : D + 1],
                )
                for t in range(t0h, t0h + H):
                    pr = scr.tile([128, 128], f32, tag="sq")
                    nc.vector.scalar_tensor_tensor(
                        out=pr[:, :],
                        in0=sb_anc[:, ts(t, 128)],
                        scalar=0.0,
                        in1=gall[:, ds(t * (D + 1), D)],
                        op0=OP.add,
                        op1=OP.mult,
                        accum_out=qcol[:, t : t + 1],
                    )
                    nc.vector.tensor_copy(
                        out=scol[:, t : t + 1],
                        in_=gall[:, ds(t * (D + 1) + D, 1)],
                    )

            for m in range(MG):
                for t in range(RT):
                    pt = ps.tile([128, GW], f32, tag="ps")
                    for k in range(GW // 512):
                        nc.tensor.matmul(
                            pt[:, ts(k, 512)],
                            lhsT=sb_anct[:, ts(t, 128)],
                            rhs=sb_ct[:, ds(m * GW + k * 512, 512)],
                            start=True,
                            stop=True,
                        )
                    if (m + t) % 2 == 1:
                        # exp to SBUF (frees the PSUM slot at ACT-end), row
                        # sum on DVE — keeps the Scalar queue lean
                        es = scr.tile([128, GW], f32, tag="es")
                        nc.scalar.activation(
                            out=es[:, :],
                            in_=pt[:, :],
                            func=AF.Exp,
                            bias=negb[:, t : t + 1],
                            scale=INVT,
                        )
                        nc.vector.reduce_sum(
                            out=dsum[:, t, m : m + 1],
                            in_=es[:, :],
                            axis=mybir.AxisListType.X,
                        )
                    else:
                        nc.scalar.activation(
                            out=pt[:, :],
                            in_=pt[:, :],
                            func=AF.Exp,
                            bias=negb[:, t : t + 1],
                            scale=INVT,
                            accum_out=dsum[:, t, m : m + 1],
                        )
                if m == 1:
                    emit_g_phase()
                if m == 2:
                    emit_G_phase(0)
                    emit_G_phase(1)

            # ---- final per-row math on [128, RT] tiles ----
            denom = sm.tile([128, RT], f32)
            nc.vector.reduce_sum(
                out=denom[:, :], in_=dsum[:, :, :], axis=mybir.AxisListType.X
            )
            lt = sm.tile([128, RT], f32)
            lnb = sm.tile([128, 1], f32)
            nc.vector.memset(lnb[:, :], EPS - 1.0)
            nc.scalar.activation(
                out=lt[:, :], in_=denom[:, :], func=AF.Ln, bias=lnb[:, :], scale=1.0
            )
            ta = sm.tile([128, RT], f32)   # S - mp
            nc.vector.tensor_tensor(ta[:, :], scol[:, :], sb_mpr[:, :], OP.subtract)
            s2 = sm.tile([128, RT], f32)   # mp * (S - mp)
            nc.vector.tensor_tensor(s2[:, :], ta[:, :], sb_mpr[:, :], OP.mult)
            t2 = sm.tile([128, RT], f32)   # (dot_ii/T) * S
            nc.vector.scalar_tensor_tensor(
                out=t2[:, :], in0=negb[:, :], scalar=-1.0, in1=scol[:, :],
                op0=OP.mult, op1=OP.mult,
            )
            t3 = sm.tile([128, RT], f32)   # (q - dot_ii*S)/T
            nc.vector.tensor_tensor(t3[:, :], qcol[:, :], t2[:, :], OP.subtract)
            s1 = sm.tile([128, RT], f32)
            nc.vector.tensor_tensor(s1[:, :], t3[:, :], sb_mpr[:, :], OP.mult)
            gz = sm.tile([128, RT], f32)   # 1 where s2 == 0
            nc.vector.tensor_scalar(gz[:, :], s2[:, :], 0.0, None, OP.is_equal)
            s2p = sm.tile([128, RT], f32)
            nc.vector.tensor_tensor(s2p[:, :], s2[:, :], gz[:, :], OP.add)
            r2 = sm.tile([128, RT], f32)
            nc.vector.reciprocal(out=r2[:, :], in_=s2p[:, :])
            u = sm.tile([128, RT], f32)    # L*s2
            nc.vector.tensor_tensor(u[:, :], lt[:, :], s2[:, :], OP.mult)
            v = sm.tile([128, RT], f32)    # L*s2 - s1
            nc.vector.tensor_tensor(v[:, :], u[:, :], s1[:, :], OP.subtract)
            lsb = sm.tile([128, RT], f32)
            nc.vector.tensor_tensor(lsb[:, :], v[:, :], r2[:, :], OP.mult)
            nc.sync.dma_start(out=loss_d[:, :], in_=lsb[:, :])

    nc.compile()
    return nc


def _marshal(features, max_probs, labels):
    import ml_dtypes

    feats = np.ascontiguousarray(np.asarray(features, dtype=np.float32))
    mp = np.asarray(max_probs, dtype=np.float32).reshape(B)
    lab = np.asarray(labels).astype(np.float32).reshape(B)

    C = np.ascontiguousarray(feats.transpose(1, 0, 2).reshape(N, D))
    ct = np.ascontiguousarray(C.T.astype(ml_dtypes.bfloat16))   # [128, N]
    crm = np.ones((128, CHUNKS, D + 1), np.float32)
    crm[:, :, :D] = C.reshape(CHUNKS, 128, D).transpose(1, 0, 2)
    crm = np.ascontiguousarray(
        crm.reshape(128, CHUNKS * (D + 1)).astype(ml_dtypes.bfloat16)
    )

    lab_full = np.tile(lab, V)                          # [N]
    mp_full = np.tile(mp, V)
    labj = np.ascontiguousarray(lab_full.reshape(CHUNKS, 128).T)
    mpj = np.ascontiguousarray(mp_full.reshape(CHUNKS, 128).T)

    in_maps = []
    for k in range(CORES):
        r0 = k * RPC
        anct = np.ascontiguousarray(ct[:, r0 : r0 + RPC])
        anc = np.ascontiguousarray(
            C.reshape(CHUNKS, 128, D)[k * RT : (k + 1) * RT]
            .transpose(1, 0, 2)
            .reshape(128, RPC)
            .astype(ml_dtypes.bfloat16)
        )
        mpr = np.ascontiguousarray(mp_full[r0 : r0 + RPC].reshape(RT, 128).T)
        labr = np.ascontiguousarray(lab_full[r0 : r0 + RPC])
        in_maps.append(
            {
                "ct": ct,
                "crm": crm,
                "anct": anct,
                "anc": anc,
                "mpr": mpr,
                "labr": labr,
                "labj": labj,
                "mpj": mpj,
            }
        )
    return in_maps


def _run_raw(in_maps, **kw):
    from concourse.bass_utils import run_bass_kernel_spmd

    if "nc" not in _CACHE:
        _CACHE["nc"] = _build_program()
    return run_bass_kernel_spmd(
        _CACHE["nc"], in_maps, core_ids=list(range(CORES)), **kw
    )


def kernel(features, max_probs, labels):
    in_maps = _marshal(features, max_probs, labels)
    res = _run_raw(in_maps)
    # loss[p, t] on core k is the loss of row k*RPC + t*128 + p; mean covers
    # every row exactly once.
    vals = np.stack([r["loss"] for r in res.results])
    return np.asarray(vals.mean(), dtype=np.float32)

